# revision 23
# baseline (speedup 1.0000x reference)
"""Allegro-style equivariant GNN edge-network on 8 TRN2 NeuronCores.

Strategy (per sharding hint): data-parallel over edges. Each of the 8 cores
processes E/8 = 16384 edges. Weights are host-folded (w_tp / Wlin / norms
fused into single matmul weights) and replicated. Device kernel runs in
feature-major layout [features, edges] so every linear map is a TensorE
matmul with stationary weights; the equivariant tensor-product bilinears are
DVE elementwise products against replicated geometric rows.
"""

import sys

sys.path.insert(0, "/opt/trn_rl_repo")

import numpy as np
import ml_dtypes

BF = ml_dtypes.bfloat16

import concourse.bass as bass
import concourse.mybir as mybir
from concourse import bacc
from concourse.tile import TileContext
from concourse.bass_utils import run_bass_kernel_spmd

# ---- problem constants (hardcoded per spec) ----
E = 131072
NCORES = 8
EC = E // NCORES  # 16384 edges per core
C = 16
S = 64
NB = 8
TE = 16
NL = 2
RMAX = 5.0

N = 1024  # edge chunk (DVE/ACT op width); matmuls run in 512-col halves
NCHUNK = EC // N

F32 = mybir.dt.float32
BF16 = mybir.dt.bfloat16


def _Qnp():
    Q = np.zeros((5, 3, 3))
    s = 1.0 / np.sqrt(2.0)
    Q[0, 0, 1] = Q[0, 1, 0] = s
    Q[1, 1, 2] = Q[1, 2, 1] = s
    Q[2] = np.diag([-1.0, -1.0, 2.0]) / np.sqrt(6.0)
    Q[3, 0, 2] = Q[3, 2, 0] = s
    Q[4] = np.diag([1.0, -1.0, 0.0]) * s
    return Q


_Q = _Qnp()
_An = np.einsum('mij,pjk,qki->mpq', _Q, _Q, _Q)
_A = 0.5 * (_An + _An.transpose(0, 2, 1))

# ---------------------------------------------------------------------------
# Host-side weight folding
# ---------------------------------------------------------------------------


def _fold_weights(inp):
    """Returns dict name -> np.float32 array for all device weight params."""
    f = lambda a: np.ascontiguousarray(a, dtype=np.float32)
    W = {}
    s0 = 1.0 / np.sqrt(3.0 * C)
    s1 = 1.0 / np.sqrt(4.0 * C)
    s2 = 1.0 / np.sqrt(4.0 * C)

    W["We1"] = f(inp["W_e1"])                      # (24,64)
    W["be1"] = f(inp["b_e1"].reshape(S, 1))
    W["We2"] = f(inp["W_e2"])
    W["be2"] = f(inp["b_e2"].reshape(S, 1))

    # env weight sets: index 0 = embed (W_env_e), 1 = after layer 0 (Wenv[0])
    env_srcs = [
        (inp["W_env_e"], inp["b_env_e"]),
        (inp["Wenv"][0], inp["benv"][0]),
    ]
    for t, (We, be) in enumerate(env_srcs):
        w16 = We                                   # (64,16)
        W[f"Wenv16_{t}"] = f(w16)
        W[f"benv16_{t}"] = f(be.reshape(C, 1))
        w48 = np.zeros((S, 48), np.float64)
        w80 = np.zeros((S, 80), np.float64)
        b48 = np.zeros((48, 1), np.float64)
        b80 = np.zeros((80, 1), np.float64)
        for i in range(3):
            w48[:, i * C:(i + 1) * C] = We
            b48[i * C:(i + 1) * C, 0] = be
        for m in range(5):
            w80[:, m * C:(m + 1) * C] = We
            b80[m * C:(m + 1) * C, 0] = be
        W[f"Wenv48_{t}"] = f(w48)
        W[f"benv48_{t}"] = f(b48)
        W[f"Wenv80_{t}"] = f(w80)
        W[f"benv80_{t}"] = f(b80)

    for l in range(NL):
        w = np.asarray(inp["w_tp"][l], np.float64)       # (11,16)
        W0 = np.asarray(inp["Wlin0"][l], np.float64)     # (48,16)
        W1 = np.asarray(inp["Wlin1"][l], np.float64)     # (64,16)
        W2 = np.asarray(inp["Wlin2"][l], np.float64)     # (64,16)

        # n0 path: t000 (from x0), t110 (from P1diag), t220 (from P4diag)
        W[f"Wt000_{l}"] = f((w[0][:, None] * W0[0:16]) * s0)           # (16,16)
        w110 = np.zeros((48, 16), np.float64)
        for i in range(3):
            w110[i * C:(i + 1) * C] = w[4][:, None] * W0[16:32] * s0
        W[f"W110f_{l}"] = f(w110)                                       # (48,16)
        w220 = np.zeros((80, 16), np.float64)
        for m in range(5):
            w220[m * C:(m + 1) * C] = w[9][:, None] * W0[32:48] * s0
        W[f"W220f_{l}"] = f(w220)                                       # (80,16)

        if l == 0:
            # --- layer-0 specific (x = y * env algebra) ---
            # t011 merged: input x1 (rows (i,c) = env_c*d_i), out n1[(i,c')]
            w011 = np.zeros((48, 48), np.float64)
            for i in range(3):
                for c in range(C):
                    w011[i * C + c, i * C:(i + 1) * C] = w[1][c] * W1[c] * s1
            _w011 = w011
            # t101: x1 direct
            w101 = np.zeros((48, 48), np.float64)
            for i in range(3):
                for c in range(C):
                    w101[i * C + c, i * C:(i + 1) * C] = w[3][c] * W1[16 + c] * s1
            W["W01_0"] = f(_w011 + w101)
            # t121+t211 merged, input P3_j rows (m,c) = x2[(m,c)]*d_j
            for j in range(3):
                wj = np.zeros((80, 48), np.float64)
                for m in range(5):
                    for i in range(3):
                        for c in range(C):
                            wj[m * C + c, i * C:(i + 1) * C] += _Q[m, i, j] * (
                                w[6][c] * W1[32 + c] + w[8][c] * W1[48 + c]) * s1
                W[f"W1221_0_{j}"] = f(wj)
            # t022 merged: input x2 rows (m,c) = env_c*y2_m
            w022 = np.zeros((80, 80), np.float64)
            for m in range(5):
                for c in range(C):
                    w022[m * C + c, m * C:(m + 1) * C] = w[2][c] * W2[c] * s2
            _w022 = w022
            # t112: input P1_j rows (i,c) = x1[(i,c)]*d_j
            for j in range(3):
                wj = np.zeros((48, 80), np.float64)
                for i in range(3):
                    for m in range(5):
                        for c in range(C):
                            wj[i * C + c, m * C:(m + 1) * C] += (
                                _Q[m, i, j] * w[5][c] * W2[16 + c] * s2)
                W[f"W112_0_{j}"] = f(wj)
            # t202: x2 direct
            w202 = np.zeros((80, 80), np.float64)
            for m in range(5):
                for c in range(C):
                    w202[m * C + c, m * C:(m + 1) * C] = w[7][c] * W2[32 + c] * s2
            W["W02_0"] = f(_w022 + w202)
            # t222: input P4_q rows (p,c) = x2[(p,c)]*y2_q
            for q in range(5):
                wq = np.zeros((80, 80), np.float64)
                for p in range(5):
                    for m in range(5):
                        for c in range(C):
                            wq[p * C + c, m * C:(m + 1) * C] += (
                                _A[m, p, q] * w[10][c] * W2[48 + c] * s2)
                W[f"W222_0_{q}"] = f(wq)

        W[f"Wm1f_{l}"] = f(inp["Wm1"][l])                 # (80,64)
        W[f"bm1_{l}"] = f(inp["bm1"][l].reshape(S, 1))
        W[f"Wm2_{l}"] = f(inp["Wm2"][l])
        W[f"bm2_{l}"] = f(inp["bm2"][l].reshape(S, 1))

    # ---- K-stacked z-weights (layer 0) ----
    # pn01 out cols: [n1(48) | n0(16)] ; pn2 out cols: n2(80)
    def _pad(a, rows, cols):
        z = np.zeros((rows, cols), np.float64)
        z[:a.shape[0], :a.shape[1]] = a
        return z

    w1221 = [np.asarray(W[f"W1221_0_{j}"], np.float64) for j in range(3)]
    w112 = [np.asarray(W[f"W112_0_{j}"], np.float64) for j in range(3)]
    w222 = [np.asarray(W[f"W222_0_{q}"], np.float64) for q in range(5)]
    w01 = np.asarray(W["W01_0"], np.float64)
    w02 = np.asarray(W["W02_0"], np.float64)
    wt000 = [np.asarray(W[f"Wt000_{l}"], np.float64) for l in range(2)]
    w110f = [np.asarray(W[f"W110f_{l}"], np.float64) for l in range(2)]
    w220f = [np.asarray(W[f"W220f_{l}"], np.float64) for l in range(2)]

    def n1col(a):   # place (K,48) into cols 0:48 of 80 (cols 48:64 pad)
        z = np.zeros((a.shape[0], 80), np.float64); z[:, 0:48] = a; return z
    def n0col(a):   # place (K,16) into cols 64:80 (32-aligned PSUM slice)
        z = np.zeros((a.shape[0], 80), np.float64); z[:, 64:80] = a; return z

    W["WSA_0"] = f(np.vstack([n1col(w1221[0]), n1col(w01)]))            # (128,64)
    W["WSB_0"] = f(np.vstack([n1col(w1221[1]), n0col(w110f[0])]))       # (128,64)
    W["WSC_0"] = f(np.vstack([n1col(w1221[2]), n0col(wt000[0])]))       # (96,64)
    W["WSD_0"] = f(n0col(w220f[0]))                                     # (80,64)
    W["WTA_0"] = f(np.vstack([w02, w112[0]]))                           # (128,80)
    W["WTB_0"] = f(np.vstack([w222[0], w112[1]]))
    W["WTC_0"] = f(np.vstack([w222[1], w112[2]]))
    W["WTD_0"] = f(np.vstack([w222[2], w222[4][0:48]]))
    W["WTE_0"] = f(np.vstack([w222[3], w222[4][48:80]]))                # (112,80)
    W["WUB_1"] = f(np.vstack([w220f[1], w110f[1]]))                     # (128,16)
    return W


def _pack_weights(W):
    """Pack all folded weights into one [128, cols] array; returns (arr, offs)
    with offs[name] = (K, M, col_off)."""
    names = list(W.keys())
    offs = {}
    col = 0
    for nm in names:
        k, m = W[nm].shape
        offs[nm] = (k, m, col)
        col += m
    arr = np.zeros((128, col), BF)
    for nm in names:
        k, m, o = offs[nm]
        arr[:k, o:o + m] = W[nm].astype(BF)
    return arr, offs


# ---------------------------------------------------------------------------
# Device kernel builder
# ---------------------------------------------------------------------------


def _build_nc(woffs, wcols):
    nc = bacc.Bacc()
    h_p = nc.declare_dram_parameter("h", [24, EC], BF16, isOutput=False)
    g_p = nc.declare_dram_parameter("geom", [8, EC], BF16, isOutput=False)
    wpack_p = nc.declare_dram_parameter("wpack", [128, wcols], BF16, isOutput=False)
    out_p = nc.declare_dram_parameter("out", [NL, S, EC], BF16, isOutput=True)

    h_ap = h_p[:]
    g_ap = g_p[:]
    out_ap = out_p[:]

    def bcast(offset, pattern):
        return bass.AP(tensor=g_ap.tensor, offset=offset, ap=pattern)

    ACT = mybir.ActivationFunctionType

    with TileContext(nc) as tc:
        with (
            tc.tile_pool(name="const", bufs=1) as constp,
            tc.tile_pool(name="work", bufs=2) as work,
            tc.tile_pool(name="psum", bufs=4, space="PSUM") as psump,
        ):
            wpack = constp.tile([128, wcols], BF16, name="wpack", tag="wpack")
            nc.sync.dma_start(out=wpack, in_=wpack_p[:])

            class _WT:
                def __getitem__(self, nm):
                    k, m, o = woffs[nm]
                    return wpack[:k, o:o + m]

            wt = _WT()

            def ps(nm):
                return psump.tile([80, N], F32, name=nm, tag="ps")

            # dummy matmul: PE observes the single weight-DMA sem first
            warmps = ps("warmps")
            nc.tensor.matmul(warmps[:1, :1], wpack[:1, :1], wpack[:1, :1],
                             start=True, stop=True)

            H = N // 512

            def mmacc(pstile, rows, terms):
                """terms = [(lhsT, rhs_tile), ...] accumulated into pstile[:rows]
                in 512-column halves."""
                nt = len(terms)
                for t, (w_, r_) in enumerate(terms):
                    for h in range(H):
                        hs = slice(h * 512, (h + 1) * 512)
                        nc.tensor.matmul(pstile[:rows, hs], w_, r_[:, hs],
                                         start=(t == 0), stop=(t == nt - 1))

            def wtile(rows, nm):
                return work.tile([rows, N], BF16, name=nm, tag=nm)

            for ch in range(NCHUNK):
                o = ch * N
                sl = slice(o, o + N)

                # full-array warmer burst: keeps the PE HAM at K=8/8 (2.4 GHz);
                # small-utilization matmuls alone never trip the activity monitor
                wp = ps("warm")
                for wi in range(12 if ch == 0 else 4):
                    nc.tensor.matmul(wp[:80, :512], wpack[:128, :80],
                                     wpack[:128, 512:1024], start=True, stop=True)

                hT = wtile(24, "hT")
                nc.sync.dma_start(out=hT, in_=h_ap[:, sl])
                drep48 = wtile(48, "drep48")
                nc.gpsimd.dma_start(
                    out=drep48, in_=bcast(o, [[EC, 3], [0, 16], [1, N]]))
                y2rep80 = wtile(80, "y2rep80")
                nc.gpsimd.dma_start(
                    out=y2rep80, in_=bcast(3 * EC + o, [[EC, 5], [0, 16], [1, N]]))
                dj3 = work.tile([80, 3 * N], BF16, name="dj3", tag="dj3")
                nc.gpsimd.dma_start(
                    out=dj3, in_=bcast(o, [[0, 80], [EC, 3], [1, N]]))
                ym5 = work.tile([80, 5 * N], BF16, name="ym5", tag="ym5")
                nc.gpsimd.dma_start(
                    out=ym5, in_=bcast(3 * EC + o, [[0, 80], [EC, 5], [1, N]]))

                def dj(j):
                    return dj3[:, j * N:(j + 1) * N]

                def ym(m):
                    return ym5[:, m * N:(m + 1) * N]

                # ---- embed MLP ----
                pe1 = ps("pe1")
                mmacc(pe1, 64, [(wt["We1"], hT)])
                sb1 = wtile(64, "sb1")
                nc.scalar.activation(sb1, pe1[:64], ACT.Silu, bias=wt["be1"])
                pe2 = ps("pe2")
                mmacc(pe2, 64, [(wt["We2"], sb1)])
                comb0 = work.tile([80, N], BF16, name="comb0", tag="comb0")
                scal0 = comb0[0:64]
                nc.scalar.activation(scal0, pe2[:64], ACT.Silu, bias=wt["be2"])

                def env_set(t, scal_t):
                    out = []
                    for nm, rows in (("16", 16), ("48", 48), ("80", 80)):
                        pv = ps(f"pv{nm}_{t}")
                        mmacc(pv, rows, [(wt[f"Wenv{nm}_{t}"], scal_t)])
                        e = wtile(rows, f"env{nm}_{t}")
                        nc.scalar.activation(e, pv[:rows], ACT.Identity,
                                             bias=wt[f"benv{nm}_{t}"])
                        out.append(e)
                    return out

                env16, env48, env80 = env_set(0, scal0)

                # ---- layer 0 (K-stacked) ----
                # stack tiles: base element TT-written in place (base partition
                # 0), fills DMA-copied in (DMA is the only partition mover)
                SA = work.tile([128, N], BF16, name="SA", tag="SA")
                SB = work.tile([128, N], BF16, name="SB", tag="SB")
                SC = work.tile([96, N], BF16, name="SC", tag="SC")
                SD = work.tile([80, N], BF16, name="SD", tag="SD")
                TA = work.tile([128, N], BF16, name="TA", tag="TA")
                TB = work.tile([128, N], BF16, name="TB", tag="TB")
                TC = work.tile([128, N], BF16, name="TC", tag="TC")
                TD = work.tile([128, N], BF16, name="TD", tag="TD")
                TE = work.tile([112, N], BF16, name="TE", tag="TE")

                x2 = TA[0:80]
                nc.vector.tensor_mul(x2, env80, y2rep80)
                x1 = wtile(48, "x1")
                nc.vector.tensor_mul(x1, env48, drep48)

                # products (in place where possible)
                nc.vector.tensor_mul(SA[0:80], x2, dj(0))    # P3_0
                nc.vector.tensor_mul(SB[0:80], x2, dj(1))    # P3_1
                nc.vector.tensor_mul(SC[0:80], x2, dj(2))    # P3_2
                nc.vector.tensor_mul(SD[0:80], x2, y2rep80)  # P4d
                nc.vector.tensor_mul(TB[0:80], x2, ym(0))    # P4_0
                nc.vector.tensor_mul(TC[0:80], x2, ym(1))    # P4_1
                nc.vector.tensor_mul(TD[0:80], x2, ym(2))    # P4_2
                nc.vector.tensor_mul(TE[0:80], x2, ym(3))    # P4_3
                P44 = wtile(80, "P44")
                nc.vector.tensor_mul(P44, x2, ym(4))         # P4_4
                P1d = wtile(48, "P1d")
                nc.vector.tensor_mul(P1d, x1, drep48)
                P1 = []
                for j in range(3):
                    t = wtile(48, f"P1_{j}")
                    nc.vector.tensor_mul(t, x1, dj(j)[:48])
                    P1.append(t)

                # fills
                nc.sync.dma_start(out=SA[80:128], in_=x1)
                nc.sync.dma_start(out=SB[80:128], in_=P1d)
                nc.sync.dma_start(out=SC[80:96], in_=env16)
                nc.sync.dma_start(out=TA[80:128], in_=P1[0])
                nc.sync.dma_start(out=TB[80:128], in_=P1[1])
                nc.sync.dma_start(out=TC[80:128], in_=P1[2])
                nc.sync.dma_start(out=TD[80:128], in_=P44[0:48])
                nc.sync.dma_start(out=TE[80:112], in_=P44[48:80])

                # pn01: cols [n1(48) | n0(16)]
                pn01 = ps("pn01")
                mmacc(pn01, 80, [(wt["WSA_0"], SA), (wt["WSB_0"], SB),
                                 (wt["WSC_0"], SC), (wt["WSD_0"], SD)])
                n1sb = wtile(48, "n1sb")
                nc.vector.tensor_copy(n1sb, pn01[0:48])
                n0s64 = work.tile([80, N], BF16, name="n0s64", tag="n0s64")
                nc.vector.tensor_copy(n0s64[64:80], pn01[64:80])
                n0sb = wtile(16, "n0sb")
                nc.sync.dma_start(out=n0sb, in_=n0s64[64:80])

                pn2 = ps("pn2")
                mmacc(pn2, 80, [(wt["WTA_0"], TA), (wt["WTB_0"], TB),
                                (wt["WTC_0"], TC), (wt["WTD_0"], TD),
                                (wt["WTE_0"], TE)])
                n2sb = wtile(80, "n2sb")
                nc.vector.tensor_copy(n2sb, pn2[:80])

                # MLP (layer 0): comb0 = [scal0 | n0]
                nc.sync.dma_start(out=comb0[64:80], in_=n0sb)
                pm1 = ps("pm1")
                mmacc(pm1, 64, [(wt["Wm1f_0"], comb0)])
                mh0 = wtile(64, "mh0")
                nc.scalar.activation(mh0, pm1[:64], ACT.Silu, bias=wt["bm1_0"])
                pm2 = ps("pm2")
                mmacc(pm2, 64, [(wt["Wm2_0"], mh0)])
                comb1 = work.tile([80, N], BF16, name="comb1", tag="comb1")
                scal1 = comb1[0:64]
                nc.scalar.activation(scal1, pm2[:64], ACT.Identity, bias=wt["bm2_0"])
                nc.sync.dma_start(out=out_ap[0, :, sl], in_=scal1)

                # ---- layer 1 (only n0 + MLP) ----
                env16b, env48b, env80b = env_set(1, scal1)
                UB = work.tile([128, N], BF16, name="UB", tag="UB")
                UC = work.tile([16, N], BF16, name="UC", tag="UC")
                x2b = wtile(80, "x2b")
                nc.vector.tensor_mul(x2b, n2sb, env80b)
                x1b = wtile(48, "x1b")
                nc.vector.tensor_mul(x1b, n1sb, env48b)
                nc.vector.tensor_mul(UC[0:16], n0sb, env16b)   # x0b in place
                nc.vector.tensor_mul(UB[0:80], x2b, y2rep80)   # P4db in place
                P1db = wtile(48, "P1db")
                nc.vector.tensor_mul(P1db, x1b, drep48)
                nc.sync.dma_start(out=UB[80:128], in_=P1db)

                pn0b = ps("pn0b")
                mmacc(pn0b, 16, [(wt["WUB_1"], UB), (wt["Wt000_1"], UC)])
                n0b = wtile(16, "n0b")
                nc.vector.tensor_copy(n0b, pn0b[:16])

                # MLP (layer 1): comb1 = [scal1 | n0b]
                nc.sync.dma_start(out=comb1[64:80], in_=n0b)
                pm1b = ps("pm1b")
                mmacc(pm1b, 64, [(wt["Wm1f_1"], comb1)])
                mh1 = wtile(64, "mh1")
                nc.scalar.activation(mh1, pm1b[:64], ACT.Silu, bias=wt["bm1_1"])
                pm2b = ps("pm2b")
                mmacc(pm2b, 64, [(wt["Wm2_1"], mh1)])
                scal2 = wtile(64, "scal2")
                nc.scalar.activation(scal2, pm2b[:64], ACT.Identity, bias=wt["bm2_1"])
                nc.sync.dma_start(out=out_ap[1, :, sl], in_=scal2)
    nc.finalize()
    return nc


_NC_CACHE = None


def _host_prep(inputs):
    """Compute h=(radial|te) and geom=(d|y2) feature-major, plus folded weights."""
    bond_dist = np.asarray(inputs["bond_dist"], np.float32)
    bond_diff = np.asarray(inputs["bond_diff"], np.float32)
    emb = np.asarray(inputs["emb_table"], np.float32)
    Z = np.asarray(inputs["Z"]).astype(np.int64)
    ei = np.asarray(inputs["edge_index"]).astype(np.int64)

    u = bond_dist / RMAX
    n = np.arange(1, NB + 1, dtype=np.float32)
    radial = (np.sqrt(np.float32(2.0 / RMAX)) *
              np.sin(np.float32(np.pi) * n * u[:, None].astype(np.float32)) /
              bond_dist[:, None])
    cutoff = np.where(u < 1.0, 1.0 - 28.0 * u**6 + 48.0 * u**7 - 21.0 * u**8, 0.0)
    radial = (radial * cutoff[:, None].astype(np.float32)).astype(np.float32)

    d = (bond_diff / (bond_dist[:, None] + np.float32(1e-8))).astype(np.float32)
    y2 = (np.sqrt(np.float32(1.5)) *
          np.einsum('mij,ei,ej->em', _Q.astype(np.float32), d, d)).astype(np.float32)

    te = (emb[Z[ei[:, 0]]] * emb[Z[ei[:, 1]]]).astype(np.float32)

    h = np.ascontiguousarray(np.concatenate([radial, te], axis=1).T.astype(BF))
    geom = np.ascontiguousarray(np.concatenate([d, y2], axis=1).T.astype(BF))
    W = _fold_weights(inputs)
    return h, geom, W


def make_in_maps(inputs):
    global _NC_CACHE
    h, geom, W = _host_prep(inputs)
    wpack, woffs = _pack_weights(W)
    if _NC_CACHE is None:
        _NC_CACHE = _build_nc(woffs, wpack.shape[1])
    in_maps = []
    for i in range(NCORES):
        sl = slice(i * EC, (i + 1) * EC)
        m = {"h": np.ascontiguousarray(h[:, sl]),
             "geom": np.ascontiguousarray(geom[:, sl]),
             "wpack": wpack}
        in_maps.append(m)
    return in_maps


def kernel(**inputs):
    in_maps = make_in_maps(inputs)
    res = run_bass_kernel_spmd(_NC_CACHE, in_maps, list(range(NCORES))).results
    out = np.concatenate(
        [np.asarray(res[i]["out"]).astype(np.float32).transpose(2, 0, 1)
         for i in range(NCORES)], axis=0)
    return np.ascontiguousarray(out)


# revision 24
# speedup vs baseline: 1.0320x; 1.0320x over previous
"""Allegro-style equivariant GNN edge-network on 8 TRN2 NeuronCores.

Strategy (per sharding hint): data-parallel over edges. Each of the 8 cores
processes E/8 = 16384 edges. Weights are host-folded (w_tp / Wlin / norms
fused into single matmul weights) and replicated. Device kernel runs in
feature-major layout [features, edges] so every linear map is a TensorE
matmul with stationary weights; the equivariant tensor-product bilinears are
DVE elementwise products against replicated geometric rows.
"""

import sys

sys.path.insert(0, "/opt/trn_rl_repo")

import numpy as np
import ml_dtypes

BF = ml_dtypes.bfloat16

import concourse.bass as bass
import concourse.mybir as mybir
from concourse import bacc
from concourse.tile import TileContext
from concourse.bass_utils import run_bass_kernel_spmd

# ---- problem constants (hardcoded per spec) ----
E = 131072
NCORES = 8
EC = E // NCORES  # 16384 edges per core
C = 16
S = 64
NB = 8
TE = 16
NL = 2
RMAX = 5.0

N = 1024  # edge chunk (DVE/ACT op width); matmuls run in 512-col halves
NCHUNK = EC // N

F32 = mybir.dt.float32
BF16 = mybir.dt.bfloat16


def _Qnp():
    Q = np.zeros((5, 3, 3))
    s = 1.0 / np.sqrt(2.0)
    Q[0, 0, 1] = Q[0, 1, 0] = s
    Q[1, 1, 2] = Q[1, 2, 1] = s
    Q[2] = np.diag([-1.0, -1.0, 2.0]) / np.sqrt(6.0)
    Q[3, 0, 2] = Q[3, 2, 0] = s
    Q[4] = np.diag([1.0, -1.0, 0.0]) * s
    return Q


_Q = _Qnp()
_An = np.einsum('mij,pjk,qki->mpq', _Q, _Q, _Q)
_A = 0.5 * (_An + _An.transpose(0, 2, 1))

# ---------------------------------------------------------------------------
# Host-side weight folding
# ---------------------------------------------------------------------------


def _fold_weights(inp):
    """Returns dict name -> np.float32 array for all device weight params."""
    f = lambda a: np.ascontiguousarray(a, dtype=np.float32)
    W = {}
    s0 = 1.0 / np.sqrt(3.0 * C)
    s1 = 1.0 / np.sqrt(4.0 * C)
    s2 = 1.0 / np.sqrt(4.0 * C)

    W["We1"] = f(inp["W_e1"])                      # (24,64)
    W["be1"] = f(inp["b_e1"].reshape(S, 1))
    W["We2"] = f(inp["W_e2"])
    W["be2"] = f(inp["b_e2"].reshape(S, 1))

    # env weight sets: index 0 = embed (W_env_e), 1 = after layer 0 (Wenv[0])
    env_srcs = [
        (inp["W_env_e"], inp["b_env_e"]),
        (inp["Wenv"][0], inp["benv"][0]),
    ]
    for t, (We, be) in enumerate(env_srcs):
        w16 = We                                   # (64,16)
        W[f"Wenv16_{t}"] = f(w16)
        W[f"benv16_{t}"] = f(be.reshape(C, 1))
        w48 = np.zeros((S, 48), np.float64)
        w80 = np.zeros((S, 80), np.float64)
        b48 = np.zeros((48, 1), np.float64)
        b80 = np.zeros((80, 1), np.float64)
        for i in range(3):
            w48[:, i * C:(i + 1) * C] = We
            b48[i * C:(i + 1) * C, 0] = be
        for m in range(5):
            w80[:, m * C:(m + 1) * C] = We
            b80[m * C:(m + 1) * C, 0] = be
        W[f"Wenv80_{t}"] = f(w80)
        W[f"benv80_{t}"] = f(b80)
        # merged [env48 | pad16 | env16] (env16 at 32-aligned psum offset 64)
        wM = np.zeros((S, 80), np.float64)
        bM = np.zeros((80, 1), np.float64)
        wM[:, 0:48] = w48
        bM[0:48] = b48
        wM[:, 64:80] = We
        bM[64:80, 0] = be
        W[f"WenvM_{t}"] = f(wM)
        W[f"benvM_{t}"] = f(bM)

    for l in range(NL):
        w = np.asarray(inp["w_tp"][l], np.float64)       # (11,16)
        W0 = np.asarray(inp["Wlin0"][l], np.float64)     # (48,16)
        W1 = np.asarray(inp["Wlin1"][l], np.float64)     # (64,16)
        W2 = np.asarray(inp["Wlin2"][l], np.float64)     # (64,16)

        # n0 path: t000 (from x0), t110 (from P1diag), t220 (from P4diag)
        W[f"Wt000_{l}"] = f((w[0][:, None] * W0[0:16]) * s0)           # (16,16)
        w110 = np.zeros((48, 16), np.float64)
        for i in range(3):
            w110[i * C:(i + 1) * C] = w[4][:, None] * W0[16:32] * s0
        W[f"W110f_{l}"] = f(w110)                                       # (48,16)
        w220 = np.zeros((80, 16), np.float64)
        for m in range(5):
            w220[m * C:(m + 1) * C] = w[9][:, None] * W0[32:48] * s0
        W[f"W220f_{l}"] = f(w220)                                       # (80,16)

        if l == 0:
            # --- layer-0 specific (x = y * env algebra) ---
            # t011 merged: input x1 (rows (i,c) = env_c*d_i), out n1[(i,c')]
            w011 = np.zeros((48, 48), np.float64)
            for i in range(3):
                for c in range(C):
                    w011[i * C + c, i * C:(i + 1) * C] = w[1][c] * W1[c] * s1
            _w011 = w011
            # t101: x1 direct
            w101 = np.zeros((48, 48), np.float64)
            for i in range(3):
                for c in range(C):
                    w101[i * C + c, i * C:(i + 1) * C] = w[3][c] * W1[16 + c] * s1
            W["W01_0"] = f(_w011 + w101)
            # t121+t211 merged, input P3_j rows (m,c) = x2[(m,c)]*d_j
            for j in range(3):
                wj = np.zeros((80, 48), np.float64)
                for m in range(5):
                    for i in range(3):
                        for c in range(C):
                            wj[m * C + c, i * C:(i + 1) * C] += _Q[m, i, j] * (
                                w[6][c] * W1[32 + c] + w[8][c] * W1[48 + c]) * s1
                W[f"W1221_0_{j}"] = f(wj)
            # t022 merged: input x2 rows (m,c) = env_c*y2_m
            w022 = np.zeros((80, 80), np.float64)
            for m in range(5):
                for c in range(C):
                    w022[m * C + c, m * C:(m + 1) * C] = w[2][c] * W2[c] * s2
            _w022 = w022
            # t112: input P1_j rows (i,c) = x1[(i,c)]*d_j
            for j in range(3):
                wj = np.zeros((48, 80), np.float64)
                for i in range(3):
                    for m in range(5):
                        for c in range(C):
                            wj[i * C + c, m * C:(m + 1) * C] += (
                                _Q[m, i, j] * w[5][c] * W2[16 + c] * s2)
                W[f"W112_0_{j}"] = f(wj)
            # t202: x2 direct
            w202 = np.zeros((80, 80), np.float64)
            for m in range(5):
                for c in range(C):
                    w202[m * C + c, m * C:(m + 1) * C] = w[7][c] * W2[32 + c] * s2
            W["W02_0"] = f(_w022 + w202)
            # t222: input P4_q rows (p,c) = x2[(p,c)]*y2_q
            for q in range(5):
                wq = np.zeros((80, 80), np.float64)
                for p in range(5):
                    for m in range(5):
                        for c in range(C):
                            wq[p * C + c, m * C:(m + 1) * C] += (
                                _A[m, p, q] * w[10][c] * W2[48 + c] * s2)
                W[f"W222_0_{q}"] = f(wq)

        W[f"Wm1f_{l}"] = f(inp["Wm1"][l])                 # (80,64)
        W[f"bm1_{l}"] = f(inp["bm1"][l].reshape(S, 1))
        W[f"Wm2_{l}"] = f(inp["Wm2"][l])
        W[f"bm2_{l}"] = f(inp["bm2"][l].reshape(S, 1))

    # ---- K-stacked z-weights (layer 0) ----
    # pn01 out cols: [n1(48) | n0(16)] ; pn2 out cols: n2(80)
    def _pad(a, rows, cols):
        z = np.zeros((rows, cols), np.float64)
        z[:a.shape[0], :a.shape[1]] = a
        return z

    w1221 = [np.asarray(W[f"W1221_0_{j}"], np.float64) for j in range(3)]
    w112 = [np.asarray(W[f"W112_0_{j}"], np.float64) for j in range(3)]
    w222 = [np.asarray(W[f"W222_0_{q}"], np.float64) for q in range(5)]
    w01 = np.asarray(W["W01_0"], np.float64)
    w02 = np.asarray(W["W02_0"], np.float64)
    wt000 = [np.asarray(W[f"Wt000_{l}"], np.float64) for l in range(2)]
    w110f = [np.asarray(W[f"W110f_{l}"], np.float64) for l in range(2)]
    w220f = [np.asarray(W[f"W220f_{l}"], np.float64) for l in range(2)]

    def n1col(a):   # place (K,48) into cols 0:48 of 80 (cols 48:64 pad)
        z = np.zeros((a.shape[0], 80), np.float64); z[:, 0:48] = a; return z
    def n0col(a):   # place (K,16) into cols 64:80 (32-aligned PSUM slice)
        z = np.zeros((a.shape[0], 80), np.float64); z[:, 64:80] = a; return z

    W["WSA_0"] = f(np.vstack([n1col(w1221[0]), n1col(w01)]))            # (128,64)
    W["WSB_0"] = f(np.vstack([n1col(w1221[1]), n0col(w110f[0])]))       # (128,64)
    W["WSC_0"] = f(np.vstack([n1col(w1221[2]), n0col(wt000[0])]))       # (96,64)
    W["WSD_0"] = f(n0col(w220f[0]))                                     # (80,64)
    W["WTA_0"] = f(np.vstack([w02, w112[0]]))                           # (128,80)
    W["WTB_0"] = f(np.vstack([w222[0], w112[1]]))
    W["WTC_0"] = f(np.vstack([w222[1], w112[2]]))
    W["WTD_0"] = f(np.vstack([w222[2], w222[4][0:48]]))
    W["WTE_0"] = f(np.vstack([w222[3], w222[4][48:80]]))                # (112,80)
    W["WUB_1"] = f(np.vstack([w220f[1], w110f[1]]))                     # (128,16)
    return W


def _pack_weights(W):
    """Pack all folded weights into one [128, cols] array; returns (arr, offs)
    with offs[name] = (K, M, col_off)."""
    names = list(W.keys())
    offs = {}
    col = 0
    for nm in names:
        k, m = W[nm].shape
        offs[nm] = (k, m, col)
        col += m
    arr = np.zeros((128, col), BF)
    for nm in names:
        k, m, o = offs[nm]
        arr[:k, o:o + m] = W[nm].astype(BF)
    return arr, offs


# ---------------------------------------------------------------------------
# Device kernel builder
# ---------------------------------------------------------------------------


def _build_nc(woffs, wcols):
    nc = bacc.Bacc()
    h_p = nc.declare_dram_parameter("h", [24, EC], BF16, isOutput=False)
    g_p = nc.declare_dram_parameter("geom", [8, EC], BF16, isOutput=False)
    wpack_p = nc.declare_dram_parameter("wpack", [128, wcols], BF16, isOutput=False)
    out_p = nc.declare_dram_parameter("out", [NL, S, EC], BF16, isOutput=True)

    h_ap = h_p[:]
    g_ap = g_p[:]
    out_ap = out_p[:]

    def bcast(offset, pattern):
        return bass.AP(tensor=g_ap.tensor, offset=offset, ap=pattern)

    ACT = mybir.ActivationFunctionType

    with TileContext(nc) as tc:
        with (
            tc.tile_pool(name="const", bufs=1) as constp,
            tc.tile_pool(name="work", bufs=2) as work,
            tc.tile_pool(name="psum", bufs=4, space="PSUM") as psump,
        ):
            wpack = constp.tile([128, wcols], BF16, name="wpack", tag="wpack")
            nc.sync.dma_start(out=wpack, in_=wpack_p[:])

            class _WT:
                def __getitem__(self, nm):
                    k, m, o = woffs[nm]
                    return wpack[:k, o:o + m]

            wt = _WT()

            def ps(nm):
                return psump.tile([80, N], F32, name=nm, tag="ps")

            # dummy matmul: PE observes the single weight-DMA sem first
            warmps = ps("warmps")
            nc.tensor.matmul(warmps[:1, :1], wpack[:1, :1], wpack[:1, :1],
                             start=True, stop=True)

            H = N // 512

            def mmacc(pstile, rows, terms):
                """terms = [(lhsT, rhs_tile), ...] accumulated into pstile[:rows]
                in 512-column halves."""
                nt = len(terms)
                for t, (w_, r_) in enumerate(terms):
                    for h in range(H):
                        hs = slice(h * 512, (h + 1) * 512)
                        nc.tensor.matmul(pstile[:rows, hs], w_, r_[:, hs],
                                         start=(t == 0), stop=(t == nt - 1))

            def wtile(rows, nm):
                return work.tile([rows, N], BF16, name=nm, tag=nm)

            for ch in range(NCHUNK):
                o = ch * N
                sl = slice(o, o + N)

                hT = wtile(24, "hT")
                nc.sync.dma_start(out=hT, in_=h_ap[:, sl])
                drep48 = wtile(48, "drep48")
                nc.gpsimd.dma_start(
                    out=drep48, in_=bcast(o, [[EC, 3], [0, 16], [1, N]]))
                y2rep80 = wtile(80, "y2rep80")
                nc.gpsimd.dma_start(
                    out=y2rep80, in_=bcast(3 * EC + o, [[EC, 5], [0, 16], [1, N]]))
                dj3 = work.tile([80, 3 * N], BF16, name="dj3", tag="dj3")
                nc.gpsimd.dma_start(
                    out=dj3, in_=bcast(o, [[0, 80], [EC, 3], [1, N]]))
                ym5 = work.tile([80, 5 * N], BF16, name="ym5", tag="ym5")
                nc.gpsimd.dma_start(
                    out=ym5, in_=bcast(3 * EC + o, [[0, 80], [EC, 5], [1, N]]))

                def dj(j):
                    return dj3[:, j * N:(j + 1) * N]

                def ym(m):
                    return ym5[:, m * N:(m + 1) * N]

                # ---- embed MLP ----
                pe1 = ps("pe1")
                mmacc(pe1, 64, [(wt["We1"], hT)])
                sb1 = wtile(64, "sb1")
                nc.scalar.activation(sb1, pe1[:64], ACT.Silu, bias=wt["be1"])
                pe2 = ps("pe2")
                mmacc(pe2, 64, [(wt["We2"], sb1)])
                comb0 = work.tile([80, N], BF16, name="comb0", tag="comb0")
                scal0 = comb0[0:64]
                nc.scalar.activation(scal0, pe2[:64], ACT.Silu, bias=wt["be2"])

                def env_set(t, scal_t):
                    pvM = ps(f"pvM_{t}")
                    mmacc(pvM, 80, [(wt[f"WenvM_{t}"], scal_t)])
                    eM = wtile(80, f"envM_{t}")
                    nc.scalar.activation(eM, pvM[:80], ACT.Identity,
                                         bias=wt[f"benvM_{t}"])
                    pv80 = ps(f"pv80_{t}")
                    mmacc(pv80, 80, [(wt[f"Wenv80_{t}"], scal_t)])
                    e80 = wtile(80, f"env80_{t}")
                    nc.scalar.activation(e80, pv80[:80], ACT.Identity,
                                         bias=wt[f"benv80_{t}"])
                    # env48 = eM[0:48], env16 = eM[64:80] (base-64)
                    return eM[64:80], eM[0:48], e80

                env16, env48, env80 = env_set(0, scal0)

                # ---- layer 0 (K-stacked) ----
                # stack tiles: base element TT-written in place (base partition
                # 0), fills DMA-copied in (DMA is the only partition mover)
                SA = work.tile([128, N], BF16, name="SA", tag="SA")
                SB = work.tile([128, N], BF16, name="SB", tag="SB")
                SC = work.tile([96, N], BF16, name="SC", tag="SC")
                SD = work.tile([80, N], BF16, name="SD", tag="SD")
                TA = work.tile([128, N], BF16, name="TA", tag="TA")
                TB = work.tile([128, N], BF16, name="TB", tag="TB")
                TC = work.tile([128, N], BF16, name="TC", tag="TC")
                TD = work.tile([128, N], BF16, name="TD", tag="TD")
                TE = work.tile([112, N], BF16, name="TE", tag="TE")

                x2 = TA[0:80]
                nc.vector.tensor_mul(x2, env80, y2rep80)
                x1 = wtile(48, "x1")
                nc.vector.tensor_mul(x1, env48, drep48)

                # products (in place where possible)
                nc.vector.tensor_mul(SA[0:80], x2, dj(0))    # P3_0
                nc.vector.tensor_mul(SB[0:80], x2, dj(1))    # P3_1
                nc.vector.tensor_mul(SC[0:80], x2, dj(2))    # P3_2
                nc.vector.tensor_mul(SD[0:80], x2, y2rep80)  # P4d
                nc.vector.tensor_mul(TB[0:80], x2, ym(0))    # P4_0
                nc.vector.tensor_mul(TC[0:80], x2, ym(1))    # P4_1
                nc.vector.tensor_mul(TD[0:80], x2, ym(2))    # P4_2
                nc.vector.tensor_mul(TE[0:80], x2, ym(3))    # P4_3
                P44 = wtile(80, "P44")
                nc.vector.tensor_mul(P44, x2, ym(4))         # P4_4
                P1d = wtile(48, "P1d")
                nc.vector.tensor_mul(P1d, x1, drep48)
                P1 = []
                for j in range(3):
                    t = wtile(48, f"P1_{j}")
                    nc.vector.tensor_mul(t, x1, dj(j)[:48])
                    P1.append(t)

                # fills
                nc.sync.dma_start(out=SA[80:128], in_=x1)
                nc.sync.dma_start(out=SB[80:128], in_=P1d)
                nc.sync.dma_start(out=SC[80:96], in_=env16)
                nc.sync.dma_start(out=TA[80:128], in_=P1[0])
                nc.sync.dma_start(out=TB[80:128], in_=P1[1])
                nc.sync.dma_start(out=TC[80:128], in_=P1[2])
                nc.sync.dma_start(out=TD[80:128], in_=P44[0:48])
                nc.sync.dma_start(out=TE[80:112], in_=P44[48:80])

                # pn01: cols [n1(48) | n0(16)]
                pn01 = ps("pn01")
                mmacc(pn01, 80, [(wt["WSA_0"], SA), (wt["WSB_0"], SB),
                                 (wt["WSC_0"], SC), (wt["WSD_0"], SD)])
                n1sb = wtile(48, "n1sb")
                nc.vector.tensor_copy(n1sb, pn01[0:48])
                n0s64 = work.tile([80, N], BF16, name="n0s64", tag="n0s64")
                nc.vector.tensor_copy(n0s64[64:80], pn01[64:80])

                pn2 = ps("pn2")
                mmacc(pn2, 80, [(wt["WTA_0"], TA), (wt["WTB_0"], TB),
                                (wt["WTC_0"], TC), (wt["WTD_0"], TD),
                                (wt["WTE_0"], TE)])
                n2sb = wtile(80, "n2sb")
                nc.vector.tensor_copy(n2sb, pn2[:80])

                # MLP (layer 0): comb0 = [scal0 | n0]
                nc.sync.dma_start(out=comb0[64:80], in_=n0s64[64:80])
                pm1 = ps("pm1")
                mmacc(pm1, 64, [(wt["Wm1f_0"], comb0)])
                mh0 = wtile(64, "mh0")
                nc.scalar.activation(mh0, pm1[:64], ACT.Silu, bias=wt["bm1_0"])
                pm2 = ps("pm2")
                mmacc(pm2, 64, [(wt["Wm2_0"], mh0)])
                comb1 = work.tile([80, N], BF16, name="comb1", tag="comb1")
                scal1 = comb1[0:64]
                nc.scalar.activation(scal1, pm2[:64], ACT.Identity, bias=wt["bm2_0"])
                nc.sync.dma_start(out=out_ap[0, :, sl], in_=scal1)

                # ---- layer 1 (only n0 + MLP) ----
                env16b, env48b, env80b = env_set(1, scal1)
                UB = work.tile([128, N], BF16, name="UB", tag="UB")
                UC = work.tile([16, N], BF16, name="UC", tag="UC")
                x2b = wtile(80, "x2b")
                nc.vector.tensor_mul(x2b, n2sb, env80b)
                x1b = wtile(48, "x1b")
                nc.vector.tensor_mul(x1b, n1sb, env48b)
                x0b64 = wtile(80, "x0b64")
                nc.vector.tensor_mul(x0b64[64:80], n0s64[64:80], env16b)
                nc.sync.dma_start(out=UC[0:16], in_=x0b64[64:80])
                nc.vector.tensor_mul(UB[0:80], x2b, y2rep80)   # P4db in place
                P1db = wtile(48, "P1db")
                nc.vector.tensor_mul(P1db, x1b, drep48)
                nc.sync.dma_start(out=UB[80:128], in_=P1db)

                pn0b = ps("pn0b")
                mmacc(pn0b, 16, [(wt["WUB_1"], UB), (wt["Wt000_1"], UC)])
                n0b = wtile(16, "n0b")
                nc.vector.tensor_copy(n0b, pn0b[:16])

                # MLP (layer 1): comb1 = [scal1 | n0b]
                nc.sync.dma_start(out=comb1[64:80], in_=n0b)
                pm1b = ps("pm1b")
                mmacc(pm1b, 64, [(wt["Wm1f_1"], comb1)])
                mh1 = wtile(64, "mh1")
                nc.scalar.activation(mh1, pm1b[:64], ACT.Silu, bias=wt["bm1_1"])
                pm2b = ps("pm2b")
                mmacc(pm2b, 64, [(wt["Wm2_1"], mh1)])
                scal2 = wtile(64, "scal2")
                nc.scalar.activation(scal2, pm2b[:64], ACT.Identity, bias=wt["bm2_1"])
                nc.sync.dma_start(out=out_ap[1, :, sl], in_=scal2)
    nc.finalize()
    return nc


_NC_CACHE = None


def _host_prep(inputs):
    """Compute h=(radial|te) and geom=(d|y2) feature-major, plus folded weights."""
    bond_dist = np.asarray(inputs["bond_dist"], np.float32)
    bond_diff = np.asarray(inputs["bond_diff"], np.float32)
    emb = np.asarray(inputs["emb_table"], np.float32)
    Z = np.asarray(inputs["Z"]).astype(np.int64)
    ei = np.asarray(inputs["edge_index"]).astype(np.int64)

    u = bond_dist / RMAX
    n = np.arange(1, NB + 1, dtype=np.float32)
    radial = (np.sqrt(np.float32(2.0 / RMAX)) *
              np.sin(np.float32(np.pi) * n * u[:, None].astype(np.float32)) /
              bond_dist[:, None])
    cutoff = np.where(u < 1.0, 1.0 - 28.0 * u**6 + 48.0 * u**7 - 21.0 * u**8, 0.0)
    radial = (radial * cutoff[:, None].astype(np.float32)).astype(np.float32)

    d = (bond_diff / (bond_dist[:, None] + np.float32(1e-8))).astype(np.float32)
    y2 = (np.sqrt(np.float32(1.5)) *
          np.einsum('mij,ei,ej->em', _Q.astype(np.float32), d, d)).astype(np.float32)

    te = (emb[Z[ei[:, 0]]] * emb[Z[ei[:, 1]]]).astype(np.float32)

    h = np.ascontiguousarray(np.concatenate([radial, te], axis=1).T.astype(BF))
    geom = np.ascontiguousarray(np.concatenate([d, y2], axis=1).T.astype(BF))
    W = _fold_weights(inputs)
    return h, geom, W


def make_in_maps(inputs):
    global _NC_CACHE
    h, geom, W = _host_prep(inputs)
    wpack, woffs = _pack_weights(W)
    if _NC_CACHE is None:
        _NC_CACHE = _build_nc(woffs, wpack.shape[1])
    in_maps = []
    for i in range(NCORES):
        sl = slice(i * EC, (i + 1) * EC)
        m = {"h": np.ascontiguousarray(h[:, sl]),
             "geom": np.ascontiguousarray(geom[:, sl]),
             "wpack": wpack}
        in_maps.append(m)
    return in_maps


def kernel(**inputs):
    in_maps = make_in_maps(inputs)
    res = run_bass_kernel_spmd(_NC_CACHE, in_maps, list(range(NCORES))).results
    out = np.concatenate(
        [np.asarray(res[i]["out"]).astype(np.float32).transpose(2, 0, 1)
         for i in range(NCORES)], axis=0)
    return np.ascontiguousarray(out)
